# revision 9
# baseline (speedup 1.0000x reference)
"""Trainium2 Bass kernel for nn_DeepBiRNN (2-layer bidirectional LSTM).

B=32, T=1024, D=H=512, L=2, OUT=1024.

Single fused launch on 2 cores (core0 = forward direction, core1 = backward,
identical program, direction lives entirely in the input data):

  pass A: xwi1 = x^T @ Wi[0] + b[0]      (For_i over 64 row-chunks of 16 steps)
  pass B: layer-1 LSTM recurrence        (For_i over 64 chunks x 16 steps)
  pass C: xwi2 = h1^T @ Wi[1] + b[1]
  pass D: layer-2 recurrence, stores relu(h2)
  pass E: partial out = relu(h2) @ Wo_half   (contraction half: core0 rows
          0:512 of W_out, core1 rows 512:1024 -> host adds the partials)

All intermediates (xwi1, xwi2, h1, h2r) are DRAM pool tiles - nothing round
trips through the host between layers, which removes ~4GB of tunnel traffic.

Matmuls bf16 with fp32 PSUM; cell state fp32. gates^T layout: PSUM [128, 512]
with partition p = gate_dim % 128, free = (G*4 + hb)*32 + batch, G in
(i, f, o, g) order, hb = gate_dim // 128.
"""

import numpy as np
import ml_dtypes

import concourse.bacc as bacc
import concourse.mybir as mybir
import concourse.tile as tile
from concourse.bass_utils import run_bass_kernel_spmd

BF16 = ml_dtypes.bfloat16
B, T, D, H = 32, 1024, 512, 512
G4 = 4 * H
RC = 64          # row chunks
TS = 16          # steps per chunk
NCORES = 2

_cache = {}


def build_fused():
    nc = bacc.Bacc("TRN2", target_bir_lowering=False, debug=False,
                   num_devices=NCORES)
    dt = mybir.dt
    # x^T tiles: [kc, 128, rc, 16t, 32b]
    xT = nc.dram_tensor("xT", [RC, 4, 128, TS, B], dt.bfloat16,
                        kind="ExternalInput")
    # recurrent weights both layers, 64 stationary tiles each
    wh = nc.dram_tensor("wh", [2, 128, 64 * 128], dt.bfloat16,
                        kind="ExternalInput")
    # input-gemm weights both layers (4kc x 16m tiles)
    wi = nc.dram_tensor("wi", [2, 128, 64 * 128], dt.bfloat16,
                        kind="ExternalInput")
    # out-gemm weights: contraction half, 4kc x 8ot tiles
    wo = nc.dram_tensor("wo", [128, 32 * 128], dt.bfloat16,
                        kind="ExternalInput")
    ident = nc.dram_tensor("ident", [128, 128], dt.bfloat16,
                           kind="ExternalInput")
    bcol = nc.dram_tensor("bcol", [128, 32], dt.float32,
                          kind="ExternalInput")  # per-layer per-m bias col
    h0 = nc.dram_tensor("h0", [2, 128, 128], dt.bfloat16, kind="ExternalInput")
    c0 = nc.dram_tensor("c0", [2, 128, 128], dt.float32, kind="ExternalInput")
    out = nc.dram_tensor("out", [RC, 8, 128, TS, B], dt.bfloat16,
                         kind="ExternalOutput")

    with tile.TileContext(nc) as tc:
        with (
            tc.tile_pool(name="const", bufs=1) as constp,
            tc.tile_pool(name="state", bufs=1) as statep,
            tc.tile_pool(name="mv", bufs=3) as mvp,
            tc.tile_pool(name="ob", bufs=3) as obp,
            tc.tile_pool(name="cell", bufs=2) as cellp,
            tc.tile_pool(name="ps", bufs=2, space="PSUM") as psp,
            tc.tile_pool(name="dram", bufs=1, space="DRAM") as dramp,
        ):
            # ---- persistent SBUF weights
            wh_sb = [constp.tile([128, 64 * 128], dt.bfloat16, tag=f"wh{l}",
                                 name=f"wh_sb{l}")
                     for l in range(2)]
            wi_sb = [constp.tile([128, 64 * 128], dt.bfloat16, tag=f"wi{l}",
                                 name=f"wi_sb{l}")
                     for l in range(2)]
            for l in range(2):
                nc.sync.dma_start(wh_sb[l][:], wh.ap()[l])
                nc.sync.dma_start(wi_sb[l][:], wi.ap()[l])
            wo_sb = constp.tile([128, 32 * 128], dt.bfloat16)
            nc.sync.dma_start(wo_sb[:], wo.ap())
            id_sb = constp.tile([128, 128], dt.bfloat16)
            nc.sync.dma_start(id_sb[:], ident.ap())
            bc_sb = constp.tile([128, 32], dt.float32)
            zcol = constp.tile([128, 1], dt.float32)
            nc.vector.memset(zcol[:], 0.0)
            nc.sync.dma_start(bc_sb[:], bcol.ap())

            # ---- DRAM intermediates
            xwi_d = [dramp.tile([RC, 128, 16, TS, B], dt.bfloat16,
                                tag=f"xwi{l}", name=f"xwi_d{l}")
                     for l in range(2)]
            h1_d = dramp.tile([RC, 4, 128, TS, B], dt.bfloat16, tag="h1")
            h2r_d = dramp.tile([RC, 4, 128, TS, B], dt.bfloat16, tag="h2r")

            # ---------------- input gemm pass: src[kc] x wi[l] -> xwi_d[l]
            def gemm_pass(l, src_slices):
                # src_slices(j, kc) -> AP [128, TS*B] moving operand
                with tc.For_i(0, RC, 1, name=f"gm{l}") as j:
                    mvs = []
                    for kc in range(4):
                        mv = mvp.tile([128, TS, B], dt.bfloat16,
                                      tag=f"gmv{kc}")
                        nc.gpsimd.dma_start(mv[:], src_slices(j, kc))
                        mvs.append(mv)
                    for m in range(16):
                        ps = psp.tile([128, TS, B], dt.float32, tag="gps")
                        for kc in range(4):
                            nc.tensor.matmul(
                                ps[:],
                                wi_sb[l][:, (m * 4 + kc) * 128:
                                         (m * 4 + kc + 1) * 128],
                                mvs[kc][:],
                                start=(kc == 0), stop=(kc == 3),
                            )
                        ob = obp.tile([128, TS, B], dt.bfloat16, tag="gob")
                        nc.vector.tensor_scalar_add(
                            ob[:], ps[:],
                            bc_sb[:, l * 16 + m:l * 16 + m + 1])
                        nc.gpsimd.dma_start(xwi_d[l][j, :, m], ob[:])

            # ---------------- recurrence pass
            def rec_pass(l, hstore):
                h_sb = statep.tile([128, 128], dt.bfloat16, tag=f"h{l}")
                nc.sync.dma_start(h_sb[:], h0.ap()[l])
                c_sb = statep.tile([128, 128], dt.float32, tag=f"c{l}")
                nc.sync.dma_start(c_sb[:], c0.ap()[l])
                with tc.For_i(0, RC, 1, name=f"rec{l}",
                              hint_engines=(mybir.EngineType.PE,)) as j:
                    for u in range(TS):
                        xw = mvp.tile([128, 16, B], dt.bfloat16, tag="xw")
                        nc.gpsimd.dma_start(xw[:], xwi_d[l][j, :, :, u])
                        ps = psp.tile([128, 512], dt.float32, tag="gates")
                        nc.tensor.matmul(ps[:], id_sb[:], xw[:],
                                         start=True, stop=False)
                        for gh in range(16):
                            for k in range(4):
                                idx = gh * 4 + k
                                nc.tensor.matmul(
                                    ps[:, gh * 32:(gh + 1) * 32],
                                    wh_sb[l][:, idx * 128:(idx + 1) * 128],
                                    h_sb[:, k * 32:(k + 1) * 32],
                                    start=False, stop=(k == 3),
                                )
                        sig = cellp.tile([128, 384], dt.float32, tag="sig")
                        nc.scalar.activation(
                            sig[:], ps[:, 0:384],
                            mybir.ActivationFunctionType.Sigmoid)
                        tg = cellp.tile([128, 128], dt.float32, tag="tg")
                        nc.scalar.activation(
                            tg[:], ps[:, 384:512],
                            mybir.ActivationFunctionType.Tanh)
                        u_t = cellp.tile([128, 128], dt.float32, tag="u")
                        nc.vector.tensor_mul(u_t[:], sig[:, 0:128], tg[:])
                        v_t = cellp.tile([128, 128], dt.float32, tag="v")
                        nc.vector.tensor_mul(v_t[:], sig[:, 128:256], c_sb[:])
                        nc.vector.tensor_add(c_sb[:], u_t[:], v_t[:])
                        th = cellp.tile([128, 128], dt.float32, tag="th")
                        nc.scalar.activation(
                            th[:], c_sb[:], mybir.ActivationFunctionType.Tanh)
                        nc.gpsimd.tensor_mul(h_sb[:], sig[:, 256:384], th[:])
                        if l == 0:
                            hs = h_sb
                        else:
                            hs = cellp.tile([128, 128], dt.bfloat16, tag="hr")
                            nc.vector.tensor_scalar_max(hs[:], h_sb[:], zcol[:, 0:1])
                        for hb in range(4):
                            nc.gpsimd.dma_start(
                                hstore[j, hb, :, u],
                                hs[:, hb * 32:(hb + 1) * 32])

            # ---------------- passes
            gemm_pass(0, lambda j, kc: xT.ap()[j, kc])
            rec_pass(0, h1_d)
            gemm_pass(1, lambda j, kc: h1_d[j, kc])
            rec_pass(1, h2r_d)

            # out-gemm: out[ot] += wo[kc,ot].T @ h2r[kc]  (contraction half)
            with tc.For_i(0, RC, 1, name="outg") as j:
                mvs = []
                for kc in range(4):
                    mv = mvp.tile([128, TS, B], dt.bfloat16, tag=f"omv{kc}")
                    nc.gpsimd.dma_start(mv[:], h2r_d[j, kc])
                    mvs.append(mv)
                for ot in range(8):
                    ps = psp.tile([128, TS, B], dt.float32, tag="ops")
                    for kc in range(4):
                        nc.tensor.matmul(
                            ps[:],
                            wo_sb[:, (ot * 4 + kc) * 128:
                                  (ot * 4 + kc + 1) * 128],
                            mvs[kc][:],
                            start=(kc == 0), stop=(kc == 3),
                        )
                    ob = obp.tile([128, TS, B], dt.bfloat16, tag="oob")
                    nc.vector.tensor_copy(ob[:], ps[:])
                    nc.gpsimd.dma_start(out.ap()[j, ot], ob[:])
    nc.compile()
    return nc


# ------------------------------------------------------------- host helpers
def to_bf(x):
    return np.ascontiguousarray(x.astype(np.float32).astype(BF16))


def pack_wh(Wh):
    """Wh [512, 2048] -> [128, 64*128] tiles (G,hb,k), G order i,f,o,g."""
    # jax gate order in memory: i, f, g, o -> ours i, f, o, g
    w = Wh.reshape(4, 128, 4, 512)            # k, p, Gsrc, 512
    w = w[:, :, [0, 1, 3, 2]]                 # -> i, f, o, g
    w = w.reshape(4, 128, 4, 4, 128)          # k, p, G, hb, pc
    w = w.transpose(2, 3, 0, 1, 4)            # G, hb, k, p, pc
    return to_bf(w.reshape(64, 128, 128).transpose(1, 0, 2).reshape(
        128, 64 * 128))


def pack_wi(Wi):
    """Wi [512, 2048] -> [128, 64*128] tiles ordered (m, kc), m=(G,hb)."""
    w = Wi.reshape(4, 128, 4, 4, 128)         # kc, p, Gsrc, hb, pc
    w = w[:, :, [0, 1, 3, 2]]                 # -> i, f, o, g
    w = w.transpose(2, 3, 0, 1, 4)            # G, hb, kc, p, pc
    return to_bf(w.reshape(16, 4, 128, 128).reshape(64, 128, 128)
                 .transpose(1, 0, 2).reshape(128, 64 * 128))


def pack_wo(Wo_half):
    """Wo_half [512, 1024] -> [128, 32*128] tiles ordered (ot, kc)."""
    w = Wo_half.reshape(4, 128, 8, 128)       # kc, p, ot, pc
    w = w.transpose(2, 0, 1, 3)               # ot, kc, p, pc
    return to_bf(w.reshape(32, 128, 128).transpose(1, 0, 2).reshape(
        128, 32 * 128))


def pack_bcol(b):
    """b [2, 2048] -> [128, 32] per-(l,m) bias columns (G order i,f,o,g)."""
    x = b.reshape(2, 4, 4, 128)               # l, Gsrc, hb, p
    x = x[:, [0, 1, 3, 2]]                    # i, f, o, g
    return np.ascontiguousarray(
        x.transpose(3, 0, 1, 2).reshape(128, 32).astype(np.float32))


def pack_state(a):
    """[B, H] -> [128, 4*32] layout [p, hb*32+b]."""
    return a.T.reshape(4, 128, B).transpose(1, 0, 2).reshape(128, 128)


def kernel(x, h0, c0, Wi_f, Wh_f, b_f, Wi_b, Wh_b, b_b, W_out, b_out):
    x = np.asarray(x, np.float32)
    h0 = np.asarray(h0, np.float32); c0 = np.asarray(c0, np.float32)
    Wi_f = np.asarray(Wi_f, np.float32); Wh_f = np.asarray(Wh_f, np.float32)
    Wi_b = np.asarray(Wi_b, np.float32); Wh_b = np.asarray(Wh_b, np.float32)
    b_f = np.asarray(b_f, np.float32); b_b = np.asarray(b_b, np.float32)
    W_out = np.asarray(W_out, np.float32); b_out = np.asarray(b_out, np.float32)

    if "fused" not in _cache:
        _cache["fused"] = build_fused()
    nc = _cache["fused"]

    # x^T [64rc, 4kc, 128, 16t, 32b]; bwd core gets time-reversed x.
    # Cast to bf16 first so the transpose copies move half the bytes.
    x16 = x.astype(BF16)                            # [32, 1024, 512]
    xt = x16.transpose(2, 1, 0)                     # [512, 1024, 32] view
    xT_f = np.ascontiguousarray(
        xt.reshape(4, 128, RC, TS, B).transpose(2, 0, 1, 3, 4))
    xT_b = np.ascontiguousarray(
        xt[:, ::-1].reshape(4, 128, RC, TS, B).transpose(2, 0, 1, 3, 4))

    ident = to_bf(np.eye(128, dtype=np.float32))
    maps = []
    for core in range(NCORES):
        fwd = core == 0
        Wh_l, Wi_l, b_l = (Wh_f, Wi_f, b_f) if fwd else (Wh_b, Wi_b, b_b)
        wo_half = W_out[0:512] if fwd else W_out[512:1024]
        maps.append({
            "xT": xT_f if fwd else xT_b,
            "wh": np.stack([pack_wh(Wh_l[0]), pack_wh(Wh_l[1])]),
            "wi": np.stack([pack_wi(Wi_l[0]), pack_wi(Wi_l[1])]),
            "wo": pack_wo(wo_half),
            "ident": ident,
            "bcol": pack_bcol(b_l),
            "h0": np.stack([to_bf(pack_state(h0[l])) for l in range(2)]),
            "c0": np.stack([
                np.ascontiguousarray(pack_state(c0[l]).astype(np.float32))
                for l in range(2)]),
        })

    import time as _time
    t0 = _time.time()
    res = run_bass_kernel_spmd(nc, maps, core_ids=list(range(NCORES)))
    _cache.setdefault("launch_times", []).append(_time.time() - t0)
    if getattr(res, "exec_time_ns", None):
        _cache["exec_time_ns"] = res.exec_time_ns

    # out partials [RC, 8ot, 128, TS*B] bf16; full = fwd + reversed(bwd)
    # partials [RC, 8ot, 128, TS, B] bf16; sum fwd + time-reversed bwd and
    # emit [B, T, 1024] fp32 with a single output-sized copy per step.
    pf = np.asarray(res.results[0]["out"])          # bf16, t ascending
    pb = np.asarray(res.results[1]["out"])[::-1, :, :, ::-1]  # reverse t
    acc = pf.astype(np.float32)
    acc += pb.astype(np.float32)
    if b_out.any():
        acc += b_out.reshape(8, 128, 1, 1)
    # [RC, 8, 128, TS, B] -> [B, RC, TS, 8, 128] -> [B, T, 1024]
    out = acc.transpose(4, 0, 3, 1, 2).reshape(B, T, 1024)
    return np.ascontiguousarray(out)
